# revision 17
# baseline (speedup 1.0000x reference)
"""Causal self-attention (RoPE) on 8 trn2 NeuronCores.

Sharding: tensor-parallel over heads. Each core owns 2 of the 16 heads:
 - Wqkv column-shard (its heads' q/k/v columns), Wproj row-shard.
 - Every core reads all of x (host pre-packed per 512-row tile so each
   (partition, tile) DMA segment is one contiguous 8KB run), computes
   qkv+RoPE+attention for its heads, and a partial projection [B*T, C].
 - Host un-shards by summing the 8 partials (the "all-reduce") + bproj.

Device-side layout (d on partitions, t on free):
 - qkv natural [t,384] = x @ Wqkv_local via stationary x^T blocks.
 - RoPE on DVE/gpsimd in natural layout, PE-transpose q,k -> qT/kT.
 - scores^T[k,q] = kT_blk.T @ qT (two heads row-tiled concurrently).
 - p^T = exp(scores^T/8) on ACT (no max subtraction: |s| < ~8, f32-safe).
 - attn out^T[d,q] += v_aug.T @ p^T, v_aug = [v | ones]; ones column
   yields the softmax denominator in row 64 for free.
 - normalize y^T by DMA-broadcast reciprocal denominators.
 - proj: out[q,:] = y^T.T @ Wproj_local, interleaved as PE filler.

Schedule: software-pipelined at 128-row granularity. Attention for
(b,qt) pair p runs at step p+1 while qkv for tile p+1 and projection
for pair p-2 drain as background work between attention j-iterations.
Each row-block's PE transposes are deferred one block so the RoPE
vector-chain latency hides under the next block's qkv matmuls.
"""

from collections import deque

import ml_dtypes
import numpy as np

import concourse.bacc as bacc
import concourse.bass as bass
import concourse.mybir as mybir
import concourse.tile as tile
from concourse.bass_utils import run_bass_kernel_spmd

F32 = mybir.dt.float32
BF16 = mybir.dt.bfloat16

B, T, C = 2, 2048, 1024
H, D = 16, 64
NCORES = 8
HL = 2                   # heads per core
R = B * T                # 4096 token rows
PB = 128                 # partition block
NQB = R // PB            # 32 row blocks total
TBB = T // PB            # 16 row blocks per batch
QT = 512                 # attention query tile
NT = R // QT             # 8 qkv tiles (512 rows each)
NQT = T // QT            # 4 attention steps per batch
KC = C // PB             # 8 contraction chunks
ROPE_BASE = 10000.0

MM_DT = BF16             # matmul operand dtype (1 cyc/row on PE)


def _build_nc():
    nc = bacc.Bacc(trn_type="TRN2")

    # host-packed layouts: every DMA row is one contiguous run
    xh = nc.dram_tensor("xh", [PB, NT * KC * QT], MM_DT, kind="ExternalInput")
    wq = nc.dram_tensor("wq", [PB, KC * 384], MM_DT, kind="ExternalInput")
    bq = nc.dram_tensor("bq", [PB, 3 * HL * D], F32, kind="ExternalInput")
    wp = nc.dram_tensor("wp", [HL * D, C], MM_DT, kind="ExternalInput")
    cosd = nc.dram_tensor("cosd", [PB, TBB * 256], MM_DT, kind="ExternalInput")
    sgn = nc.dram_tensor("sgn", [PB, TBB * 256], MM_DT, kind="ExternalInput")
    idn = nc.dram_tensor("idn", [PB, PB], MM_DT, kind="ExternalInput")
    bm2 = nc.dram_tensor("bm2", [PB, 2 * PB], MM_DT, kind="ExternalInput")
    out = nc.dram_tensor("out", [R, C], BF16, kind="ExternalOutput")
    rns = nc.dram_tensor("rns", [2 * HL, T], F32)

    with tile.TileContext(nc) as tc:
        _body(nc, tc, xh, wq, bq, wp, cosd, sgn, idn, bm2, out, rns)
    nc.finalize()
    return nc


def _body(nc, tc, xh, wq, bq, wp, cosd, sgn, idn, bm2, out, rns):
    import contextlib

    ctx = contextlib.ExitStack()
    with ctx:
        singles = ctx.enter_context(tc.tile_pool(name="singles", bufs=1))

        xh_r = xh.rearrange("p (t kc q) -> p t kc q", t=NT, kc=KC)
        xt_tiles = {}
        xt_pool = ctx.enter_context(tc.tile_pool(name="xt", bufs=3))

        def load_xt(t):
            xtt = xt_pool.tile([PB, KC, QT], MM_DT, tag="xt", name=f"xt{t}")
            if t == 0:
                # split the startup-critical first tile so qkv can begin
                # accumulating on the first half while the second lands
                nc.sync.dma_start(out=xtt[:, 0:4, :], in_=xh_r[:, t, 0:4, :])
                nc.sync.dma_start(out=xtt[:, 4:8, :], in_=xh_r[:, t, 4:8, :])
            else:
                nc.sync.dma_start(out=xtt, in_=xh_r[:, t])
            xt_tiles[t] = xtt

        # ---- input loads: x tile 0 first (critical path), weights on the
        # scalar queue in parallel, tables on the gpsimd queue -------------
        load_xt(0)
        wq_t = singles.tile([PB, KC, 384], MM_DT)
        nc.scalar.dma_start(out=wq_t, in_=wq.rearrange("p (kc n) -> p kc n", kc=KC))
        idn_t = singles.tile([PB, PB], MM_DT)
        nc.scalar.dma_start(out=idn_t, in_=idn[:, :])
        bq_t = singles.tile([PB, 384], F32)
        nc.scalar.dma_start(out=bq_t, in_=bq[:, :])
        cos_t = singles.tile([PB, TBB, 256], MM_DT)
        nc.gpsimd.dma_start(out=cos_t, in_=cosd.rearrange("p (tb c) -> p tb c", tb=TBB))
        sgn_t = singles.tile([PB, TBB, 256], MM_DT)
        nc.gpsimd.dma_start(out=sgn_t, in_=sgn.rearrange("p (tb c) -> p tb c", tb=TBB))
        bm_t = singles.tile([PB, 2, PB], MM_DT)
        nc.gpsimd.dma_start(out=bm_t, in_=bm2.rearrange("p (g x) -> p g x", g=2))
        wp_t = singles.tile([PB, C], MM_DT)
        nc.gpsimd.dma_start(out=wp_t, in_=wp[:, :])
        load_xt(1)

        # ---- resident activations ----------------------------------------
        qkT_b = [
            singles.tile([PB, TBB, 2, PB], MM_DT, name=f"qkT{b}") for b in range(B)
        ]
        va_b = [
            singles.tile([PB, HL, TBB, 65], MM_DT, name=f"va{b}") for b in range(B)
        ]
        yT_all = singles.tile([PB, R], F32)
        yT_nrm = singles.tile([PB, R], MM_DT)
        dnc = singles.tile([PB, 2 * HL * 4 * 4], F32)   # [p, pair*16+qt*4+j]
        rec = singles.tile([PB, 2 * HL * 4 * 4], F32)
        rn = singles.tile([PB, R], F32)
        ones64 = singles.tile([65, 64], F32)
        rrow = singles.tile([65, QT], F32)

        for b in range(B):
            nc.vector.memset(va_b[b], 1.0)
        nc.vector.memset(ones64, 1.0)

        with (
            tc.tile_pool(name="qkw", bufs=4) as qk_pool,
            tc.tile_pool(name="pt", bufs=6) as pt_pool,
            tc.tile_pool(name="ost", bufs=3) as ost_pool,
            tc.tile_pool(name="psb", bufs=2, space="PSUM") as psb_pool,
            tc.tile_pool(name="pss", bufs=2, space="PSUM") as pss_pool,
            tc.tile_pool(name="pso", bufs=1, space="PSUM") as pso_pool,
        ):
            # ---------------- background work machinery --------------------
            bg = deque()

            def drain(n):
                for _ in range(n):
                    if not bg:
                        return
                    bg.popleft()()

            # transposes for block n are deferred until after block n+1's
            # qkv matmuls so the RoPE DVE/gpsimd chain hides under PE work
            pend_tail = deque()

            def qkv_part(t, ql):
                qb = t * 4 + ql
                b, tb = divmod(qb, TBB)
                xt = xt_tiles[t]
                ps_qkv = psb_pool.tile([PB, 384], F32, tag="psb", name="psq")
                for kc in range(KC):
                    nc.tensor.matmul(
                        ps_qkv,
                        lhsT=xt[:, kc, ql * PB : (ql + 1) * PB],
                        rhs=wq_t[:, kc, :],
                        start=(kc == 0),
                        stop=(kc == KC - 1),
                    )
                qk_nat = qk_pool.tile([PB, 256], MM_DT, tag="qknat", name="qn")
                nc.vector.tensor_tensor(
                    out=qk_nat, in0=ps_qkv[:, 0:256], in1=bq_t[:, 0:256],
                    op=mybir.AluOpType.add,
                )
                nc.vector.tensor_tensor(
                    out=va_b[b][:, :, tb, 0:64],
                    in0=ps_qkv[:, 256:384].rearrange("p (h d) -> p h d", h=2),
                    in1=bq_t[:, 256:384].rearrange("p (h d) -> p h d", h=2),
                    op=mybir.AluOpType.add,
                )
                rtmp = qk_pool.tile([PB, 256], MM_DT, tag="rtmp", name="rt")
                qk_rot = qk_pool.tile([PB, 256], MM_DT, tag="qkrot", name="qr")
                src = bass.AP(
                    tensor=qk_nat.tensor,
                    offset=qk_nat.offset + 32,
                    ap=[list(qk_nat.ap[0]), [128, 2], [64, 2], [-32, 2], [1, 32]],
                )
                nc.gpsimd.tensor_tensor(
                    out=rtmp.rearrange("p (g h f x) -> p g h f x", g=2, h=2, f=2),
                    in0=src,
                    in1=sgn_t[:, tb, :].rearrange(
                        "p (g h f x) -> p g h f x", g=2, h=2, f=2
                    ),
                    op=mybir.AluOpType.mult,
                )
                nc.vector.tensor_tensor(
                    out=qk_rot, in0=qk_nat, in1=cos_t[:, tb, :],
                    op=mybir.AluOpType.mult,
                )
                nc.vector.tensor_tensor(
                    out=qk_rot, in0=qk_rot, in1=rtmp, op=mybir.AluOpType.add,
                )
                pend_tail.append((b, tb, qk_rot))

            def tail_part():
                b, tb, qk_rot = pend_tail.popleft()
                ps_t = psb_pool.tile([PB, 2, PB], MM_DT, tag="psb", name="pst")
                nc.tensor.transpose(ps_t[:, 0, :], qk_rot[:, 0:128], idn_t)
                nc.tensor.transpose(ps_t[:, 1, :], qk_rot[:, 128:256], idn_t)
                nc.vector.tensor_copy(qkT_b[b][:, tb, :, :], ps_t)

            def ql_unit(t, ql):
                qkv_part(t, ql)
                while len(pend_tail) > 1:
                    tail_part()

            def flush_tails():
                while pend_tail:
                    tail_part()

            proj_n = [0]

            def proj_qb(qb):
                """projection for one 128-row output block + store."""
                ot = ost_pool.tile([PB, C], BF16, tag="ot", name="ot")
                for nb in range(2):
                    ncol = slice(nb * QT, (nb + 1) * QT)
                    pp = psb_pool.tile([PB, QT], F32, tag="psb", name="pp")
                    nc.tensor.matmul(
                        pp,
                        lhsT=yT_nrm[:, qb * PB : (qb + 1) * PB],
                        rhs=wp_t[:, ncol],
                        start=True,
                        stop=True,
                    )
                    if nb == 0:
                        nc.vector.tensor_copy(ot[:, ncol], pp)
                    else:
                        nc.scalar.copy(ot[:, ncol], pp)
                proj_n[0] += 1
                nc.sync.dma_start(out=out[qb * PB : (qb + 1) * PB, :], in_=ot)

            # ---------------- attention ------------------------------------
            def attn_qt(b, qt, midcb=None):
                po = [
                    pso_pool.tile([65, QT], F32, tag=f"po{h}", name=f"po{h}")
                    for h in range(HL)
                ]
                jmax = qt * 4 + 4

                def s_off(j):
                    return max(j - qt * 4, 0) * PB

                def emit_scores(j):
                    off = s_off(j)
                    ps = pss_pool.tile([PB, HL, QT], F32, tag="pss", name="ps")
                    for h in range(HL):
                        nc.tensor.matmul(
                            ps[:, h, off:QT],
                            lhsT=qkT_b[b][h * 64 : h * 64 + 64, j, 1, :],
                            rhs=qkT_b[b][
                                h * 64 : h * 64 + 64,
                                qt * 4 + off // PB : qt * 4 + 4, 0, :,
                            ],
                            start=True,
                            stop=True,
                        )
                    return ps

                ps_cur = emit_scores(0)
                for j in range(jmax):
                    m = j - qt * 4
                    off = s_off(j)
                    pt = pt_pool.tile([PB, HL, QT], MM_DT, tag="pt", name="pt")
                    nc.scalar.activation(
                        out=pt[:, :, off:QT], in_=ps_cur[:, :, off:QT],
                        func=mybir.ActivationFunctionType.Exp, scale=0.125,
                    )
                    if m >= 0:
                        nc.gpsimd.tensor_tensor(
                            out=pt[:, :, off : off + PB],
                            in0=pt[:, :, off : off + PB],
                            in1=bm_t, op=mybir.AluOpType.mult,
                        )
                    if j + 1 < jmax:
                        ps_nxt = emit_scores(j + 1)
                    if midcb is not None and j == 4:
                        midcb()
                    drain(3 if 2 * j < jmax else 2)
                    for h in range(HL):
                        nc.tensor.matmul(
                            po[h][:, off:QT],
                            lhsT=va_b[b][:, h, j, :],
                            rhs=pt[:, h, off:QT],
                            start=(j == 0),
                            stop=(j == jmax - 1),
                        )
                    if j + 1 < jmax:
                        ps_cur = ps_nxt
                cols = slice(b * T + qt * QT, b * T + (qt + 1) * QT)
                st = [
                    pt_pool.tile([65, QT], F32, tag=f"st{h}", name=f"st{h}")
                    for h in range(HL)
                ]
                nc.vector.tensor_copy(st[0][0:65, :], po[0][0:65, :])
                nc.scalar.copy(st[1][0:65, :], po[1][0:65, :])
                nc.sync.dma_start(out=yT_all[0:64, cols], in_=st[0][0:64, :])
                nc.sync.dma_start(out=yT_all[64:128, cols], in_=st[1][0:64, :])
                for h in range(HL):
                    base = qt * 16 + (2 * b + h) * 4
                    nc.sync.dma_start(
                        out=dnc[:, base : base + 4], in_=st[h][64:65, :],
                    )
                return st

            def norm_a(b, qt):
                c0 = qt * 16 + 2 * b * 4
                nc.vector.reciprocal(rec[:, c0 : c0 + 8], dnc[:, c0 : c0 + 8])
                for h in range(HL):
                    pair = 2 * b + h
                    dst = rns[pair : pair + 1, qt * QT : (qt + 1) * QT]
                    dst = bass.AP(
                        tensor=dst.tensor, offset=dst.offset,
                        ap=[[4, 128], [1, 4]],
                    )
                    nc.sync.dma_start(
                        out=dst, in_=rec[:, c0 + 4 * h : c0 + 4 * h + 4],
                    )
                    src = rns[pair : pair + 1, qt * QT : (qt + 1) * QT]
                    src = bass.AP(
                        tensor=src.tensor,
                        offset=src.offset,
                        ap=[[0, 64]] + [list(d) for d in src.ap[1:]],
                    )
                    nc.sync.dma_start(
                        out=rn[h * 64 : h * 64 + 64,
                               b * T + qt * QT : b * T + (qt + 1) * QT],
                        in_=src,
                    )

            def norm_b2(b, qt):
                ccols = slice(b * T + qt * QT, b * T + (qt + 1) * QT)
                nc.vector.tensor_tensor(
                    out=yT_nrm[:, ccols], in0=yT_all[:, ccols], in1=rn[:, ccols],
                    op=mybir.AluOpType.mult,
                )

            def emit_proj_pair(p2):
                bp, qp = divmod(p2, NQT)
                norm_b2(bp, qp)
                for qb in range(p2 * 4, p2 * 4 + 4):
                    bg.append((lambda q=qb: proj_qb(q)))

            # ---------------- schedule -------------------------------------
            # step 0: prologue — first qkv tile inline, prefetch next
            for ql in range(4):
                ql_unit(0, ql)

            # steps 1..8: attention pair s-1 + background tile s + proj s-3
            for s in range(1, NT + 1):
                if s + 1 < NT:
                    load_xt(s + 1)
                flush_tails()
                if s < NT:
                    for ql in range(4):
                        bg.append((lambda t=s, q=ql: ql_unit(t, q)))
                if s - 3 >= 0:
                    emit_proj_pair(s - 3)
                p = s - 1           # attention pair this step
                ba, qa = divmod(p, NQT)
                # on the last (longest) step, inject pair 6's projection
                # mid-loop so only pair 7 remains for the tail
                midcb = (lambda: emit_proj_pair(NT - 2)) if s == NT else None
                st_last = attn_qt(ba, qa, midcb=midcb)
                if s < NT:
                    norm_a(ba, qa)
                    drain(len(bg))

            # tail: pair 7 normalized via PE-broadcast reciprocals (no DRAM
            # round-trip), emitted ahead of the leftover proj drains so the
            # vector queue reaches the reciprocals promptly
            ccols = slice(T + (NQT - 1) * QT, 2 * T)
            for h in range(HL):
                nc.vector.reciprocal(
                    rrow[h * 64 : h * 64 + 1, :], st_last[h][64:65, :]
                )
            rn_ps = psb_pool.tile([PB, QT], F32, tag="psb", name="rnps")
            for h in range(HL):
                nc.tensor.matmul(
                    rn_ps[h * 64 : h * 64 + 64, :],
                    lhsT=ones64[h * 64 : h * 64 + 1, :],
                    rhs=rrow[h * 64 : h * 64 + 1, :],
                    start=True,
                    stop=True,
                )
            nc.vector.tensor_tensor(
                out=yT_nrm[:, ccols], in0=yT_all[:, ccols], in1=rn_ps,
                op=mybir.AluOpType.mult,
            )
            drain(len(bg))
            for qb in range(NQB - 4, NQB):
                proj_qb(qb)


_NC_CACHE = None
LAST_RESULTS = None


def _rope_tables():
    inv = 1.0 / (ROPE_BASE ** (np.arange(0, D, 2, dtype=np.float32) / D))
    t = np.arange(T, dtype=np.float32)
    fr = np.einsum("i,j->ij", t, inv)            # [T, 32]
    emb = np.concatenate([fr, fr], axis=1)       # [T, 64]
    cos = np.cos(emb).astype(np.float32)
    sin = np.sin(emb).astype(np.float32)
    sgn = np.concatenate([-sin[:, 0:32], sin[:, 32:64]], axis=1)
    cosd = np.tile(cos, (1, 4))                  # dup for (q/k) x 2 heads
    sgnd = np.tile(sgn, (1, 4))
    # pack [T, 256] -> [128, TBB*256] so each partition row is contiguous
    cosp = np.ascontiguousarray(
        cosd.reshape(TBB, PB, 256).transpose(1, 0, 2).reshape(PB, TBB * 256)
    )
    sgnp = np.ascontiguousarray(
        sgnd.reshape(TBB, PB, 256).transpose(1, 0, 2).reshape(PB, TBB * 256)
    )
    return (
        cosp.astype(ml_dtypes.bfloat16),
        sgnp.astype(ml_dtypes.bfloat16),
    )


def kernel(x, Wqkv, bqkv, Wproj, bproj):
    global _NC_CACHE, LAST_RESULTS
    x = np.asarray(x, dtype=np.float32)
    Wqkv = np.asarray(Wqkv, dtype=np.float32)
    bqkv = np.asarray(bqkv, dtype=np.float32)
    Wproj = np.asarray(Wproj, dtype=np.float32)
    bproj = np.asarray(bproj, dtype=np.float32)

    # pack x^T [C, R] -> [128, NT, KC, 512]: per (partition, tile) one
    # contiguous 8KB DMA segment
    xT = x.reshape(R, C).T.astype(ml_dtypes.bfloat16)          # [C, R]
    xp = np.ascontiguousarray(
        xT.reshape(KC, PB, NT, QT).transpose(1, 2, 0, 3).reshape(PB, -1)
    )
    cosd, sgnd = _rope_tables()
    idn = np.eye(PB, dtype=np.float32).astype(ml_dtypes.bfloat16)
    bm0 = (np.tril(np.ones((PB, PB), dtype=np.float32))).T  # [k,u]: u>=k
    bm2 = np.ascontiguousarray(np.concatenate([bm0, bm0], axis=1)).astype(
        ml_dtypes.bfloat16
    )

    in_maps = []
    for r in range(NCORES):
        hsel = [2 * r, 2 * r + 1]
        wcols = []
        for part in range(3):  # q, k, v column groups
            for h in hsel:
                wcols.append(Wqkv[:, part * C + h * D : part * C + (h + 1) * D])
        wq_l = np.concatenate(wcols, axis=1).astype(ml_dtypes.bfloat16)
        # pack [C, 384] -> [128, KC*384]: contiguous per-partition rows
        wq_p = np.ascontiguousarray(
            wq_l.reshape(KC, PB, 384).transpose(1, 0, 2).reshape(PB, KC * 384)
        )
        bq_cols = []
        for part in range(3):
            for h in hsel:
                bq_cols.append(bqkv[part * C + h * D : part * C + (h + 1) * D])
        bq_l = np.concatenate(bq_cols)                     # [384]
        bq_b = np.ascontiguousarray(np.broadcast_to(bq_l, (PB, 384)))
        wp_l = np.ascontiguousarray(Wproj[r * PB : (r + 1) * PB, :]).astype(
            ml_dtypes.bfloat16
        )
        in_maps.append(
            {
                "xh": xp, "wq": wq_p, "bq": bq_b, "wp": wp_l,
                "cosd": cosd, "sgn": sgnd, "idn": idn, "bm2": bm2,
            }
        )

    if _NC_CACHE is None:
        _NC_CACHE = _build_nc()
    res = run_bass_kernel_spmd(_NC_CACHE, in_maps, core_ids=list(range(NCORES)))
    LAST_RESULTS = res
    acc = np.zeros((R, C), dtype=np.float32)
    for r in range(NCORES):
        acc += np.asarray(res.results[r]["out"], dtype=np.float32)
    acc += bproj[None, :]
    return acc.reshape(B, T, C)
